# revision 28
# baseline (speedup 1.0000x reference)
"""Distributed multi-head attention (QKV proj + RoPE + softmax attention + out proj)
on 8 TRN2 NeuronCores.

Sharding: tensor-parallel over heads. Core c owns heads (2c, 2c+1):
  - qkv^T = W_c @ x^T for its 384 channels over all 4096 tokens (bf16 matmul);
    per n-tile the v slice is computed FIRST so its PE transposes into the
    [v | 1] layout overlap the q/k slices instead of gating the next n-tile
  - RoPE on q,k in bf16: ScalarE evicts PSUM (+bias) to bf16, the partition
    half-swap is 4 SBUF-SBUF DMAs (split across sync/gpsimd queues), and all
    three elementwise ops run on VectorE in 2x bf16 mode with bf16 cos/sin
    tables (sign folded so the swap happens before the sin multiply)
  - scores^T = k @ q^T per (batch, head): contraction is only 64, so the two
    heads' matmuls run CONCURRENTLY in the PE array via row-tiling into one
    2-bank PSUM tile; one exp [128,1024] on ScalarE
  - ctx^T = [v | 1] @ expS^T : M=65 matmul computes context + softmax
    denominator (ones column baked into the transposed-v layout)
  - per-(qt,head) pipelined normalization (approx reciprocal + partition
    broadcast via a stride-0 DRAM read); batch-1 QKV/rope work is drip-fed
    into batch-0's attention to keep the TensorEngine warm
  - TWO AllToAlls (one per batch) redistribute ctx head-sharded ->
    token-sharded (256 tok/core/batch).  A2A(b0) fires as soon as batch-0's
    context is evicted and flies UNDER batch-1's attention compute; A2A(b1)
    is bridged by the batch-0 out-projection (its inputs - ctx_full(b0),
    W_out - are prefetched during batch 1), so no dummy matmuls are needed
  - out^T = W_out^T.T @ ctx_b^T + b_out per batch for the core's 256 tokens;
    batch-0's output columns are DMA'd out while batch 1 is still in flight

Startup: W_qkv rides the sync DMA queue while the first x n-tile rides
gpsimd, so the first matmul starts at ~3.5us; cos/sin halves follow the
weights and arrive just before the first rope multiply needs them.

Host side: transposes/shards weights, runs SPMD, gathers [1024, 512] fp32 per
core (cols 0:256 = batch-0 tokens, 256:512 = batch-1), reassembles
[2, 2048, 1024].
"""

import numpy as np
import ml_dtypes

import concourse.bass as bass
import concourse.tile as tile
from concourse import bacc, mybir
from concourse.bass_utils import run_bass_kernel_spmd
from concourse.masks import make_identity

BF16 = ml_dtypes.bfloat16

B, L, D, H, Hd = 2, 2048, 1024, 16, 64
T = B * L              # 4096 tokens
NC = 8                 # cores
HPC = H // NC          # 2 heads per core
TOK = T // NC          # 512 token shard per core (256 per batch)
TPB = TOK // B         # 256 tokens per core per batch
NT = T // 512          # 8 token n-tiles of 512
KT = L // 128          # 16 k-tiles per batch
QT = L // 512          # 4 q-tiles per batch

F32 = mybir.dt.float32
BF = mybir.dt.bfloat16


def build(debug=False):
    nc = bacc.Bacc(None, target_bir_lowering=False, num_devices=NC)

    xT = nc.dram_tensor("xT", [D, T], BF, kind="ExternalInput")          # x^T, replicated
    wq = nc.dram_tensor("wqkvT", [D, 3 * 128], BF, kind="ExternalInput")  # W_c^T per core
    bq = nc.dram_tensor("bqkv", [128, 3], F32, kind="ExternalInput")      # bias cols q,k,v
    cosT = nc.dram_tensor("cosT", [128, L], BF, kind="ExternalInput")
    sinT = nc.dram_tensor("sinT", [128, L], BF, kind="ExternalInput")    # sign-folded sin
    wo = nc.dram_tensor("woutT", [D, D], BF, kind="ExternalInput")        # W_out^T, replicated
    bo = nc.dram_tensor("bout", [128, NC], F32, kind="ExternalInput")
    out = nc.dram_tensor("out", [D, TOK], F32, kind="ExternalOutput")

    with tile.TileContext(nc) as tc:
        with tc.tile_pool(name="const", bufs=1) as const, \
             tc.tile_pool(name="big", bufs=1) as big, \
             tc.tile_pool(name="rope", bufs=3) as rope, \
             tc.tile_pool(name="es", bufs=10) as esp, \
             tc.tile_pool(name="cu", bufs=12) as cup, \
             tc.tile_pool(name="small", bufs=3) as small, \
             tc.tile_pool(name="psum", bufs=1, space="PSUM") as psum, \
             tc.tile_pool(name="dram", bufs=1, space="DRAM") as dram:

            # ---------------- constants / weights ----------------------------
            # Queue layout at startup: sync carries W_qkv then bias then cos/sin;
            # gpsimd carries the first x n-tile, so the first QKV matmul can
            # start as soon as max(W, x0) lands (~3.5us) instead of after a
            # serial 3.8MB load.
            ident = const.tile([128, 128], BF, tag="ident")
            make_identity(nc, ident[:])
            ones_bc = const.tile([1, 64], F32, tag="ones_bc")
            nc.vector.memset(ones_bc[:], 1.0)

            w_sb = []
            for k in range(8):
                t = big.tile([128, 3 * 128], BF, tag=f"w{k}", name=f"w{k}")
                nc.sync.dma_start(t[:], wq[128 * k:128 * (k + 1), :])
                w_sb.append(t)
            bq_sb = const.tile([128, 3], F32, tag="bq")
            nc.sync.dma_start(bq_sb[:], bq[:])
            bo_sb = const.tile([128, NC], F32, tag="bo")
            cos_sb = const.tile([128, L], BF, tag="cos")
            sin_sb = const.tile([128, L], BF, tag="sin")
            wo_sb = [big.tile([128, D], BF, tag=f"wo{k}", name=f"wo_{k}")
                     for k in range(8)]

            qT_sb = big.tile([128, T], BF, tag="qT")
            kT_sb = big.tile([128, T], BF, tag="kT")
            v_sb = big.tile([128, T], BF, tag="v")
            # transposed v with a built-in ones column: [tok%128, blk, head, 65]
            vn_sb = big.tile([128, T // 128, HPC, 65], BF, tag="vn")
            nc.vector.memset(vn_sb[:, :, :, 64:65], 1.0)

            # per-batch A2A buffers: [dst core, 128 ctx channels, 256 tokens]
            a2a_in = [dram.tile([NC, 128, TPB], BF, tag=f"a2a_in{b}",
                                name=f"a2a_in{b}") for b in range(B)]
            a2a_out = [dram.tile([NC, 128, TPB], BF, tag=f"a2a_out{b}",
                                 name=f"a2a_out{b}") for b in range(B)]

            # ---------------- per-stage emitters ------------------------------
            _xc_cache = {}

            def stage1_load(n, eng=None):
                ts = slice(512 * n, 512 * (n + 1))
                xc = []
                for k in range(8):
                    t = rope.tile([128, 512], BF, tag="xc", bufs=24,
                                  name=f"xc_{n}_{k}")
                    (eng or nc.sync).dma_start(t[:], xT[128 * k:128 * (k + 1), ts])
                    xc.append(t)
                _xc_cache[n] = xc

            def stage1_qkv_m(n, m):
                """QKV matmul + bias (+rope for q/k) for one (n-tile, m)."""
                ts = slice(512 * n, 512 * (n + 1))
                cs = slice(512 * (n % QT), 512 * (n % QT) + 512)
                xc = _xc_cache[n]
                if True:
                    ps = psum.tile([128, 512], F32, tag="st", bufs=3,
                                   name=f"s1_{n}_{m}")
                    for k in range(8):
                        nc.tensor.matmul(
                            ps[:],
                            w_sb[k][:, 128 * m:128 * (m + 1)],
                            xc[k][:],
                            start=(k == 0), stop=(k == 7),
                        )
                    if m < 2:  # q or k: ACT evicts (+bias, ->bf16) fast to
                        # free the PSUM slot; rope mults in bf16 on DVE (2x
                        # mode); partition-swap is ONE SBUF-SBUF DMA with a
                        # block-swapped source AP
                        dst = qT_sb if m == 0 else kT_sb
                        qb = rope.tile([128, 512], BF, tag="qb", bufs=6,
                                       name=f"qb_{n}_{m}")
                        nc.scalar.activation(
                            qb[:], ps[:],
                            mybir.ActivationFunctionType.Identity,
                            bias=bq_sb[:, m:m + 1])
                        qw = rope.tile([128, 512], BF, tag="qw", name=f"qw_{n}_{m}")
                        for blk in range(4):
                            sb = 32 * (blk ^ 1)
                            (nc.sync if blk % 2 == m else nc.gpsimd).dma_start(
                                qw[32 * blk:32 * blk + 32, :],
                                qb[sb:sb + 32, :])
                        qc = rope.tile([128, 512], BF, tag="qc", name=f"qc_{n}_{m}")
                        nc.vector.tensor_tensor(
                            qc[:], qb[:], cos_sb[:, cs], mybir.AluOpType.mult)
                        qs = rope.tile([128, 512], BF, tag="qs", name=f"qs_{n}_{m}")
                        nc.vector.tensor_tensor(
                            qs[:], qw[:], sin_sb[:, cs], mybir.AluOpType.mult)
                        nc.vector.tensor_tensor(
                            dst[:, ts], qc[:], qs[:], mybir.AluOpType.add)
                    else:  # v: bias only, straight to bf16
                        nc.scalar.activation(
                            v_sb[:, ts], ps[:],
                            mybir.ActivationFunctionType.Identity,
                            bias=bq_sb[:, 2:3])

            def stage1_qkv(n):
                stage1_load(n)
                for m in (2, 0, 1):
                    stage1_qkv_m(n, m)

            def stage1_vtr(j):
                """Transpose one 128-token block of v into vn (both heads)."""
                tp = psum.tile([128, 128], BF, tag="st", bufs=3, name=f"tr_{j}")
                nc.tensor.transpose(tp[:], v_sb[:, 128 * j:128 * (j + 1)], ident[:])
                for h in range(HPC):
                    nc.vector.tensor_copy(
                        vn_sb[:, j, h, 0:64], tp[:, 64 * h:64 * (h + 1)])

            def stage2_open(b, qt):
                return [psum.tile([65, 512], F32, tag=f"ctx{h}", bufs=1,
                                  name=f"ctx_{b}_{qt}_{h}")
                        for h in range(HPC)]

            def stage2_kts(b, qt, ctxs, kts, fill_iter):
                qsl = slice(2048 * b + 512 * qt, 2048 * b + 512 * qt + 512)
                for kt in kts:
                    ksl = slice(2048 * b + 128 * kt, 2048 * b + 128 * kt + 128)
                    blk = 16 * b + kt
                    st2 = psum.tile([128, 1024], F32, tag="st", bufs=3,
                                    name=f"st_{b}_{qt}_{kt}")
                    for h in range(HPC):
                        nc.tensor.matmul(
                            st2[:, 512 * h:512 * (h + 1)],
                            kT_sb[64 * h:64 * (h + 1), ksl],
                            qT_sb[64 * h:64 * (h + 1), qsl],
                            start=True, stop=True)
                    es = esp.tile([128, 1024], BF, tag="es",
                                  name=f"es_{b}_{qt}_{kt}")
                    nc.scalar.activation(
                        es[:], st2[:], mybir.ActivationFunctionType.Exp)
                    for h in range(HPC):
                        nc.tensor.matmul(
                            ctxs[h][:],
                            vn_sb[:, blk, h, :],
                            es[:, 512 * h:512 * (h + 1)],
                            start=(kt == 0), stop=(kt == KT - 1))
                    fill_iter(b, qt, kt)

            def stage2_qtile(b, qt, ctx_evict, fill_iter):
                ctxs = stage2_open(b, qt)
                stage2_kts(b, qt, ctxs, range(KT), fill_iter)
                ctx_evict(qt, ctxs)

            def run_batch(b, fill_iter, qts=range(QT), pre_ctxs=None):
                """Stage-2 for one batch; per-(qt,h) pipelined normalization."""

                def ctx_evict(qt, ctxs):
                    for h in range(HPC):
                        cu = cup.tile([65, 512], F32, tag="cu",
                                      name=f"cu_{b}_{qt}_{h}")
                        nc.vector.tensor_copy(cu[:], ctxs[h][:])
                        dn = small.tile([1, 512], F32, tag="dn",
                                        name=f"dn_{b}_{qt}_{h}", bufs=3)
                        nc.vector.tensor_copy(dn[:], cu[64:65, :])
                        rc = small.tile([1, 512], F32, tag="rc",
                                        name=f"rc_{b}_{qt}_{h}", bufs=3)
                        nc.vector.reciprocal_approx_fast(rc[:], dn[:])
                        dr = dram.tile([1, 512], F32, tag="dr",
                                       name=f"dr_{b}_{qt}_{h}", bufs=4)
                        nc.gpsimd.dma_start(dr[:], rc[:])
                        bca = small.tile([64, 512], F32, tag="bca",
                                         name=f"bca_{b}_{qt}_{h}", bufs=3)
                        dr_ap = dr[:]
                        bcast_src = bass.AP(
                            tensor=dr_ap.tensor, offset=dr_ap.offset,
                            ap=[[0, 32]] + [list(p) for p in dr_ap.ap])
                        nc.gpsimd.dma_start(bca[0:32, :], bcast_src)
                        nc.sync.dma_start(bca[32:64, :], bcast_src)
                        cn = small.tile([64, 512], BF, tag="cn",
                                        name=f"cn_{b}_{qt}_{h}")
                        nc.vector.tensor_tensor(
                            cn[:], cu[0:64, :], bca[:],
                            mybir.AluOpType.mult)
                        # scatter the two 256-token halves to their dst cores'
                        # slabs of this batch's A2A input
                        nc.sync.dma_start(
                            a2a_in[b][2 * qt, 64 * h:64 * (h + 1), :],
                            cn[:, 0:TPB])
                        nc.sync.dma_start(
                            a2a_in[b][2 * qt + 1, 64 * h:64 * (h + 1), :],
                            cn[:, TPB:512])

                if pre_ctxs is not None:
                    ctx_evict(0, pre_ctxs)
                for qt in qts:
                    stage2_qtile(b, qt, ctx_evict, fill_iter)
                return ctx_evict

            def emit_a2a(b):
                nc.gpsimd.collective_compute(
                    "AllToAll",
                    mybir.AluOpType.bypass,
                    replica_groups=[list(range(NC))],
                    ins=[a2a_in[b].opt()],
                    outs=[a2a_out[b].opt()],
                )

            _ctxf = {}

            def stage4_load(b, ks, eng):
                ctxf = _ctxf.setdefault(b, {})
                for k in ks:
                    t = big.tile([128, TPB], BF, tag=f"cf{b}_{k}",
                                 name=f"cf{b}_{k}")
                    eng.dma_start(t[:], a2a_out[b][k, :, :])
                    ctxf[k] = t

            def stage4_m(b, m):
                ctxf = _ctxf[b]
                pso = psum.tile([128, 512], F32, tag="st", bufs=3,
                                name=f"o{b}_{m}")
                for k in range(8):
                    nc.tensor.matmul(
                        pso[:, 0:TPB],
                        wo_sb[k][:, 128 * m:128 * (m + 1)],
                        ctxf[k][:],
                        start=(k == 0), stop=(k == 7))
                os_t = small.tile([128, TPB], F32, tag="os", name=f"os{b}_{m}")
                nc.scalar.activation(
                    os_t[:], pso[:, 0:TPB],
                    mybir.ActivationFunctionType.Identity,
                    bias=bo_sb[:, m:m + 1])
                nc.sync.dma_start(
                    out[128 * m:128 * (m + 1), TPB * b:TPB * (b + 1)], os_t[:])

            # ---------------- emission schedule -------------------------------
            # stage 1 for batch 0 (transposes follow each n-tile's v);
            # n=0's x loads ride gpsimd so they overlap the weight DMAs.
            stage1_load(0, eng=nc.gpsimd)
            nc.sync.dma_start(cos_sb[:, 0:1024], cosT[:, 0:1024])
            nc.sync.dma_start(sin_sb[:, 0:1024], sinT[:, 0:1024])
            stage1_qkv_m(0, 2)
            for j in range(0, 4):
                stage1_vtr(j)
            stage1_qkv_m(0, 0)
            stage1_qkv_m(0, 1)
            stage1_load(1)
            nc.sync.dma_start(cos_sb[:, 1024:L], cosT[:, 1024:L])
            nc.sync.dma_start(sin_sb[:, 1024:L], sinT[:, 1024:L])
            stage1_qkv_m(1, 2)
            for j in range(4, 8):
                stage1_vtr(j)
            stage1_qkv_m(1, 0)
            stage1_qkv_m(1, 1)
            for n in range(2, QT):
                stage1_load(n)
                stage1_qkv_m(n, 2)
                for j in range(4 * n, 4 * n + 4):
                    stage1_vtr(j)
                stage1_qkv_m(n, 0)
                stage1_qkv_m(n, 1)

            # stage 2 for batch 0, with stage-1(b=1) units drip-fed to keep PE busy
            b1_units = []
            for n in range(QT, NT):
                b1_units.append(lambda n=n: stage1_load(n))
                b1_units.append(lambda n=n: stage1_qkv_m(n, 2))
                for j in range(4 * n, 4 * n + 4):
                    b1_units.append(lambda j=j: stage1_vtr(j))
                b1_units.append(lambda n=n: stage1_qkv_m(n, 0))
                b1_units.append(lambda n=n: stage1_qkv_m(n, 1))
            unit_idx = [0]
            count = [0]
            # 64 kt-iterations in batch 0; 32 fill units -> every 2nd iteration
            def fill_iter(b, qt, kt):
                count[0] += 1
                if b == 0 and count[0] % 2 == 0 and unit_idx[0] < len(b1_units):
                    b1_units[unit_idx[0]]()
                    unit_idx[0] += 1

            run_batch(0, fill_iter)
            # batch-0 context is fully evicted: fire its A2A right behind the
            # last a2a_in write on the same (sync) queue -> zero-wait doorbell.
            emit_a2a(0)
            for k in range(8):
                nc.sync.dma_start(wo_sb[k][:], wo[128 * k:128 * (k + 1), :])
            nc.sync.dma_start(bo_sb[:], bo[:])
            while unit_idx[0] < len(b1_units):
                b1_units[unit_idx[0]]()
                unit_idx[0] += 1

            # batch-1 attention; A2A(b0) flies underneath it.  ctx_full(b0) is
            # prefetched once the collective has had time to complete.
            b1_fills = []
            b1_fills.append(lambda: stage4_load(0, range(0, 4), nc.sync))
            b1_fills.append(lambda: stage4_load(0, range(4, 8), nc.sync))
            fidx = [0]

            def fill_b1(b, qt, kt):
                # after ~2.5 qtiles (~40us into batch 1) the A2A(b0) is done
                if (qt, kt) in ((2, 8), (2, 12)):
                    b1_fills[fidx[0]]()
                    fidx[0] += 1

            run_batch(1, fill_b1)

            # ---------------- stage 4: out projections ------------------------
            # batch-0 out-projection is emitted before the A2A(b1) doorbell so
            # its matmuls (deps: ctx_full(b0) prefetch + W_out, both long done)
            # bridge the A2A(b1) flight window.  The doorbell rides gpsimd,
            # which stage4 never touches, so it fires as soon as batch-1's
            # a2a_in writes land.
            for m in range(8):
                stage4_m(0, m)
            emit_a2a(1)
            stage4_load(1, range(0, 8, 2), nc.sync)
            stage4_load(1, range(1, 8, 2), nc.gpsimd)
            for m in range(8):
                stage4_m(1, m)

    nc.compile()
    return nc


_NC_CACHE = None


def _get_nc():
    global _NC_CACHE
    if _NC_CACHE is None:
        _NC_CACHE = build()
    return _NC_CACHE


def _host_prep(x, W_qkv, b_qkv, W_out, b_out):
    x = np.asarray(x, dtype=np.float32)
    W_qkv = np.asarray(W_qkv, dtype=np.float32)
    b_qkv = np.asarray(b_qkv, dtype=np.float32)
    W_out = np.asarray(W_out, dtype=np.float32)
    b_out = np.asarray(b_out, dtype=np.float32)

    scale = 1.0 / np.sqrt(Hd)
    xT = np.ascontiguousarray(x.reshape(T, D).T).astype(BF16)

    # rope tables (token position within batch), channel-transposed + sign-folded
    inv_freq = 1.0 / (10000.0 ** (np.arange(0, Hd, 2, dtype=np.float32) / Hd))  # [32]
    t_pos = np.arange(L, dtype=np.float32)
    freqs = np.outer(t_pos, inv_freq)                       # [L, 32]
    emb = np.concatenate([freqs, freqs], axis=1)            # [L, 64]
    cos_t = np.cos(emb).T.astype(np.float32)                # [64, L]
    sin_t = np.sin(emb).T.astype(np.float32)                # [64, L]
    sin2 = sin_t.copy()
    sin2[:32, :] *= -1.0                    # swap-first: s''[d] = -sin d<32
    cosT = np.ascontiguousarray(np.tile(cos_t, (2, 1))).astype(BF16)   # [128, L]
    sinT = np.ascontiguousarray(np.tile(sin2, (2, 1))).astype(BF16)

    woutT = np.ascontiguousarray(W_out.T).astype(BF16)      # [D, D]
    bo_sb = np.ascontiguousarray(b_out.reshape(NC, 128).T)  # [128, 8]

    in_maps = []
    for c in range(NC):
        r = slice(128 * c, 128 * (c + 1))
        Wq = W_qkv[0 * D:1 * D][r] * scale
        Wk = W_qkv[1 * D:2 * D][r]
        Wv = W_qkv[2 * D:3 * D][r]
        Wc = np.concatenate([Wq, Wk, Wv], axis=0)           # [384, 1024]
        WcT = np.ascontiguousarray(Wc.T).astype(BF16)       # [1024, 384]
        bq_c = np.stack([
            b_qkv[0 * D:1 * D][r] * scale,
            b_qkv[1 * D:2 * D][r],
            b_qkv[2 * D:3 * D][r],
        ], axis=1).astype(np.float32)                       # [128, 3]
        in_maps.append({
            "xT": xT,
            "wqkvT": WcT,
            "bqkv": np.ascontiguousarray(bq_c),
            "cosT": cosT,
            "sinT": sinT,
            "woutT": woutT,
            "bout": bo_sb,
        })
    return in_maps


def kernel_run(inputs, trace=False, tmpdir=None):
    nc = _get_nc()
    in_maps = _host_prep(**inputs)
    res = run_bass_kernel_spmd(
        nc, in_maps, list(range(NC)), trace=trace, tmpdir=tmpdir)
    # per-core result [1024, 512]: cols 0:256 batch-0 tokens, 256:512 batch-1
    outT = np.empty((D, T), dtype=np.float32)               # [1024, 4096]
    for c in range(NC):
        r = np.asarray(res.results[c]["out"], dtype=np.float32)
        outT[:, TPB * c:TPB * (c + 1)] = r[:, 0:TPB]
        outT[:, L + TPB * c:L + TPB * (c + 1)] = r[:, TPB:TOK]
    out = np.ascontiguousarray(outT.T).reshape(B, L, D)
    return out, res


def kernel(**inputs):
    out, _ = kernel_run(inputs, trace=False)
    return out
